# revision 70
# baseline (speedup 1.0000x reference)
# Causal self-attention (B=2, T=2048, D=1024, H=16, HD=64) with RoPE on 8 TRN2
# cores.
#
# Sharding: data-parallel over batch (2 groups of 4 cores), tensor-parallel
# over heads within each group (4 heads per core, as 2 head-pairs p=0,1).
# Everything on-device is bf16 (inputs pre-converted on host): bf16 matmuls run
# at 1 cycle/row at ANY moving size (no fp32r >=256 constraint), DVE
# elementwise ops get the 2x packed mode, and DMA bytes are halved.
#
# Per core:
#   Phase A - stream x by 512-col t-chunks; for each chunk accumulate the six
#     128-col qkv projections (q/k/v x 2 pairs) over 8 contraction tiles.
#     t0/t1 interleave the contraction tiles across all six outputs (DMA-
#     paced); t2/t3 run output-major with immediate per-output drains so the
#     PSUM banks hand over to attention without a bubble. Drains: RoPE (Act
#     psum->bf16 copy + DVE shuffle/mul/mul/add, 2x packed mode) for q/k,
#     PE transposes + copies into the AV-stationary layout for v (ones
#     column appended by memset -> softmax denominator comes free out of the
#     AV matmul). Act enters attention with no backlog.
#   Attention - one continuous software pipeline over all (q-strip, head-
#     pair) strips in order 3,0,1,2 with S emitted 3 units ahead of AV so
#     the PE never waits on exp; exp on the Scalar engine (the only engine
#     with transcendentals - the attention bottleneck at ~73us, so all other
#     work stays off it); fully-causal block pairs share one exp call per
#     head; the causal mask is a constant -1e9 lower-triangle tile added
#     into the S psum group by a 128-row matmul, so exp masks for free and
#     nothing sits between exp and AV. AV accumulates [65, q] (row 64 =
#     denominator), then per-head reciprocal / partition-broadcast / mul
#     into oT.
#   Out-projection - row-sharded partial [D, T]; chunks sprinkled between
#     attention units (reusing the S psum tag) to fill the PE while Act
#     catches up on exp. Host sums the 4 partials per batch and transposes.
import sys
import os

sys.path.insert(0, "/opt/trn_rl_repo")

import numpy as np

import concourse.bass as bass  # noqa: F401  (bass types used via bacc)
import concourse.mybir as mybir
from concourse import bacc
from concourse.tile import TileContext
from concourse.bass_utils import run_bass_kernel_spmd
from contextlib import ExitStack

F32 = mybir.dt.float32
BF16 = mybir.dt.bfloat16
AF = mybir.ActivationFunctionType
ALU = mybir.AluOpType

B, T, D = 2, 2048, 1024
H, HD = 16, 64
NCORES = 8
GROUPS = NCORES // B          # cores per batch = 4
HPC = H // GROUPS             # heads per core = 4
NK = D // 128                 # contraction tiles for D
SCALE = HD ** -0.5

# hd interleave: new row 2j <- orig j, new row 2j+1 <- orig j+32 so the
# rotate-half partner of every row is its neighbour (swappable by a 32-lane
# stream shuffle).
PI = np.empty(HD, dtype=np.int64)
PI[0::2] = np.arange(32)
PI[1::2] = np.arange(32, 64)

SWAP_MASK = []
for _i in range(16):
    SWAP_MASK += [2 * _i + 1, 2 * _i]

# w_cat column tiles (host order): c0=q pair0, c1=k pair0 (first so pair-0
# attention inputs drain earliest, and the first w DMA can cover just c0/c1),
# c2/c3 = v pairs, c4=q pair1, c5=k pair1.
ROPE_DST = {0: ("q", 0), 1: ("k", 0), 4: ("q", 1), 5: ("k", 1)}


def _build_program():
    nc = bacc.Bacc("TRN2", target_bir_lowering=False, debug=False,
                   num_devices=NCORES)
    d_x = nc.dram_tensor("xT", [D, T], BF16, kind="ExternalInput").ap()
    d_w = nc.dram_tensor("w_cat", [D, 6 * 128], BF16,
                         kind="ExternalInput").ap()
    d_wo = nc.dram_tensor("w_o", [2 * 128, D], BF16,
                          kind="ExternalInput").ap()
    d_cos = nc.dram_tensor("cos2", [128, T], BF16, kind="ExternalInput").ap()
    d_sin = nc.dram_tensor("sin2", [128, T], BF16, kind="ExternalInput").ap()
    d_id = nc.dram_tensor("ident", [128, 128], BF16,
                          kind="ExternalInput").ap()
    # [si*4+j, r, nn*512+q] blocks; host reassembles to [D, T]
    d_out = nc.dram_tensor("outp", [16, 128, 1024], BF16,
                           kind="ExternalOutput").ap()
    dbg = bool(int(os.environ.get("KDEBUG", "0")))
    if dbg:
        d_dbg_q0 = nc.dram_tensor("dbg_q0", [128, T], BF16,
                                  kind="ExternalOutput").ap()
        d_dbg_k0 = nc.dram_tensor("dbg_k0", [128, T], BF16,
                                  kind="ExternalOutput").ap()
        d_dbg_va0 = nc.dram_tensor("dbg_va0", [128, 16 * 65], BF16,
                                   kind="ExternalOutput").ap()
        d_dbg_o0 = nc.dram_tensor("dbg_o0", [128, T], BF16,
                                  kind="ExternalOutput").ap()

    with TileContext(nc) as tc, nc.allow_low_precision(reason="bf16 attn"):
        with ExitStack() as root:
            qkv_pool = root.enter_context(tc.tile_pool(name="qkv", bufs=1))
            va_pool = root.enter_context(tc.tile_pool(name="va", bufs=1))
            out_pool = root.enter_context(tc.tile_pool(name="outT", bufs=1))
            wop = root.enter_context(tc.tile_pool(name="wop", bufs=1))
            wu_pool = root.enter_context(tc.tile_pool(name="wu", bufs=1))

            qT = [qkv_pool.tile([128, T], BF16, tag=f"q{p}", name=f"qT{p}")
                  for p in range(2)]
            kT = [qkv_pool.tile([128, T], BF16, tag=f"k{p}", name=f"kTt{p}")
                  for p in range(2)]
            va = [va_pool.tile([128, 16 * 65], BF16, tag=f"va{h}",
                               name=f"va{h}") for h in range(HPC)]
            oT = [out_pool.tile([128, T], BF16, tag=f"o{p}", name=f"oT{p}")
                  for p in range(2)]
            wo_sb = [wop.tile([128, D], BF16, tag=f"wo{p}", name=f"wo{p}")
                     for p in range(2)]

            # Warm the Act engine's exp table before it matters.
            wu = wu_pool.tile([1, 2], F32, tag="wu")
            wu2 = wu_pool.tile([1, 2], F32, tag="wu2")
            nc.vector.memset(wu[:], 0.0)
            nc.scalar.activation(wu2[:], wu[:], AF.Exp, scale=1.0)
            # denominator ones columns (free on Pool; avoids a strided DMA)
            for h in range(HPC):
                nc.gpsimd.memset(va[h][:, 64:16 * 65:65], 1.0)
            # causal-mask tile: -1e9 where col < partition, else 0. Added
            # into the S psum group via one tiny matmul (stationary =
            # identity) so exp zeroes the future positions exactly and no
            # per-block affine_select sits on the exp->AV critical path.
            tri = wu_pool.tile([128, 128], BF16, tag="tri")
            nc.gpsimd.memset(tri[:], -1e9)
            # keep -1e9 where p - j - 1 >= 0 (col j < partition p), else 0
            nc.gpsimd.affine_select(tri[:], tri[:], pattern=[[-1, 128]],
                                    compare_op=ALU.is_ge, fill=0.0,
                                    base=-1, channel_multiplier=1)

            # ---------------- Phase A: qkv projection + RoPE + v transpose
            with nc.named_scope("qkv"):
                with ExitStack() as sA:
                    tab = sA.enter_context(tc.tile_pool(name="tab", bufs=1))
                    xp = sA.enter_context(tc.tile_pool(name="xp", bufs=1))
                    wp = sA.enter_context(tc.tile_pool(name="wp", bufs=1))
                    tp = sA.enter_context(tc.tile_pool(name="ropetmp",
                                                       bufs=2))
                    vtp = sA.enter_context(tc.tile_pool(name="vT", bufs=1))

                    cos2 = tab.tile([128, T], BF16, tag="cos")
                    sin2 = tab.tile([128, T], BF16, tag="sin")
                    ident = wu_pool.tile([128, 128], BF16, tag="id")
                    vT = [vtp.tile([128, T], BF16, tag=f"v{p}",
                                   name=f"vT{p}") for p in range(2)]
                    x_sb = [xp.tile([128, T], BF16, tag=f"x{kt}",
                                    name=f"xsb{kt}") for kt in range(NK)]
                    w_sb = [wp.tile([128, 6 * 128], BF16, tag=f"w{kt}",
                                    name=f"wsb{kt}") for kt in range(NK)]

                    # DMA queue: x(kt,t0) + w(kt) pairs stream first (the
                    # first w transfer covers only q0/k0 so the PE starts
                    # ~2us in); tables follow in consumption order.
                    for kt in range(NK):
                        if kt == 0:
                            nc.sync.dma_start(out=x_sb[0][:, 0:256],
                                              in_=d_x[0:128, 0:256])
                            nc.sync.dma_start(out=w_sb[0][:, 0:256],
                                              in_=d_w[0:128, 0:256])
                            nc.sync.dma_start(out=x_sb[0][:, 256:512],
                                              in_=d_x[0:128, 256:512])
                        else:
                            nc.sync.dma_start(
                                out=x_sb[kt][:, 0:512],
                                in_=d_x[kt * 128:(kt + 1) * 128, 0:512])
                        if kt == 0:
                            nc.sync.dma_start(out=w_sb[0][:, 256:768],
                                              in_=d_w[0:128, 256:768])
                            nc.sync.dma_start(out=cos2[:, 0:512],
                                              in_=d_cos[:, 0:512])
                            nc.sync.dma_start(out=sin2[:, 0:512],
                                              in_=d_sin[:, 0:512])
                        else:
                            nc.sync.dma_start(
                                out=w_sb[kt][:],
                                in_=d_w[kt * 128:(kt + 1) * 128, :])
                    nc.sync.dma_start(out=cos2[:, 512:1024],
                                      in_=d_cos[:, 512:1024])
                    nc.sync.dma_start(out=sin2[:, 512:1024],
                                      in_=d_sin[:, 512:1024])
                    nc.sync.dma_start(out=ident[:], in_=d_id[:])
                    for kt in range(NK):
                        nc.sync.dma_start(
                            out=x_sb[kt][:, 512:1024],
                            in_=d_x[kt * 128:(kt + 1) * 128, 512:1024])
                    for kt in range(NK):
                        nc.sync.dma_start(
                            out=x_sb[kt][:, 1024:2048],
                            in_=d_x[kt * 128:(kt + 1) * 128, 1024:2048])
                    nc.sync.dma_start(out=cos2[:, 1024:2048],
                                      in_=d_cos[:, 1024:2048])
                    nc.sync.dma_start(out=sin2[:, 1024:2048],
                                      in_=d_sin[:, 1024:2048])
                    for p in range(2):
                        nc.sync.dma_start(
                            out=wo_sb[p][:],
                            in_=d_wo[p * 128:(p + 1) * 128, :])

                    accp = tc.alloc_tile_pool(name="accs", bufs=1,
                                              space="PSUM")
                    psT = tc.alloc_tile_pool(name="psT", bufs=2, space="PSUM",
                                             side="right")

                    def emit_rope(c, acc, tsl):
                        kind, pair = ROPE_DST[c]
                        dst = qT[pair] if kind == "q" else kT[pair]
                        qsb = tp.tile([128, 512], BF16, tag="qsb")
                        nc.scalar.copy(qsb[:], acc[:])
                        # StreamShuffle cannot convert dtypes, so shuffle
                        # the staged bf16 copy rather than the f32 psum
                        qsh = tp.tile([128, 512], BF16, tag="qsh")
                        nc.vector.stream_shuffle(qsh[:], qsb[:], SWAP_MASK)
                        tcos = tp.tile([128, 512], BF16, tag="tcos")
                        nc.vector.tensor_tensor(out=tcos[:], in0=qsb[:],
                                                in1=cos2[:, tsl],
                                                op=ALU.mult)
                        nc.vector.tensor_tensor(out=qsh[:], in0=qsh[:],
                                                in1=sin2[:, tsl],
                                                op=ALU.mult)
                        nc.vector.tensor_tensor(out=dst[:, tsl], in0=tcos[:],
                                                in1=qsh[:], op=ALU.add)

                    def emit_vtrans(t):
                        # transposes + va copies for both v c-tiles of
                        # chunk t. GPSIMD cannot touch PSUM, so these go to
                        # Act (idle in phase A) except t3 -> DVE so Act
                        # enters attention with no backlog
                        if t == 3:
                            cp = nc.vector.tensor_copy
                        else:
                            cp = nc.scalar.copy
                        for p in range(2):
                            pt_ = psT.tile([128, 512], BF16, tag="pt",
                                           name=f"ptr{p}_{t}")
                            for j in range(4):
                                tt = 4 * t + j
                                nc.tensor.transpose(
                                    pt_[:, j * 128:(j + 1) * 128],
                                    vT[p][:, tt * 128:(tt + 1) * 128],
                                    ident[:])
                            for j in range(4):
                                tt = 4 * t + j
                                cp(va[2 * p][:, tt * 65:tt * 65 + 64],
                                   pt_[:, j * 128:j * 128 + 64])
                                cp(va[2 * p + 1][:, tt * 65:tt * 65 + 64],
                                   pt_[:, j * 128 + 64:j * 128 + 128])

                    def drain(c, acc, tsl):
                        if c in (2, 3):
                            nc.scalar.copy(vT[c - 2][:, tsl], acc[:])
                        else:
                            emit_rope(c, acc, tsl)

                    for t in range(4):
                        tsl = slice(t * 512, (t + 1) * 512)
                        accs = [accp.tile([128, 512], F32, tag=f"a{c}",
                                          name=f"acc{c}_{t}")
                                for c in range(6)]
                        if t < 2:
                            # contraction-tile inner: matches the x DMA pace
                            for kt in range(NK):
                                for c in range(6):
                                    nc.tensor.matmul(
                                        accs[c][:],
                                        w_sb[kt][:, c * 128:(c + 1) * 128],
                                        x_sb[kt][:, tsl],
                                        start=(kt == 0), stop=(kt == NK - 1))
                            if t == 1:
                                emit_vtrans(0)
                            for c in range(6):
                                drain(c, accs[c], tsl)
                        else:
                            # output-major with immediate drains: PSUM banks
                            # free progressively, so attention starts with no
                            # bubble after t=3.
                            # t=3 orders the slow RoPE drains first and the
                            # quick v copies last, so the final PSUM acc
                            # frees (and attention's pools allocate) sooner
                            corder = [0, 1, 4, 5, 2, 3] if t == 3 \
                                else list(range(6))
                            for ci, c in enumerate(corder):
                                for kt in range(NK):
                                    nc.tensor.matmul(
                                        accs[c][:],
                                        w_sb[kt][:, c * 128:(c + 1) * 128],
                                        x_sb[kt][:, tsl],
                                        start=(kt == 0), stop=(kt == NK - 1))
                                if ci == 5:
                                    emit_vtrans(t - 1)
                                drain(c, accs[c], tsl)
                    emit_vtrans(3)
                    psT.release()
                    accp.release()

            # ---------------- attention + interleaved out-projection
            psS = tc.alloc_tile_pool(name="psS", bufs=2, space="PSUM")
            psV = tc.alloc_tile_pool(name="psV", bufs=2, space="PSUM",
                                     side="right")

            with nc.named_scope("attn"):
                with ExitStack() as sB:
                    ptp = sB.enter_context(tc.tile_pool(name="ptp", bufs=8))
                    rp = sB.enter_context(tc.tile_pool(name="rp", bufs=3))
                    fop = sB.enter_context(tc.tile_pool(name="fop", bufs=6))

                    # oproj chunks of the previous strip, sprinkled between
                    # attention units to fill the PE while the Act engine
                    # (the attention bottleneck) catches up on exp
                    oproj_q = []
                    divs_done = {}

                    def emit_oproj_chunk(si, j, tail=False):
                        q0 = 512 * si
                        pD = psS.tile([128, 1024], F32, tag="sps",
                                      name=f"pD{si}_{j}")
                        for nn in range(2):
                            n = 2 * j + nn
                            for p in range(2):
                                nc.tensor.matmul(
                                    pD[:, nn * 512:(nn + 1) * 512],
                                    wo_sb[p][:, n * 128:(n + 1) * 128],
                                    oT[p][:, q0:q0 + 512],
                                    start=(p == 0), stop=(p == 1))
                        fo = fop.tile([128, 1024], BF16, tag="fo",
                                      name=f"fo{si}_{j}")
                        if tail:
                            # drain halves on DVE + Act in parallel with
                            # per-half DMAs to shorten the final chain
                            nc.vector.tensor_copy(fo[:, 0:512],
                                                  pD[:, 0:512])
                            nc.sync.dma_start(
                                out=d_out[4 * si + j][:, 0:512],
                                in_=fo[:, 0:512])
                            nc.scalar.copy(fo[:, 512:1024],
                                           pD[:, 512:1024])
                            nc.sync.dma_start(
                                out=d_out[4 * si + j][:, 512:1024],
                                in_=fo[:, 512:1024])
                        else:
                            nc.vector.tensor_copy(fo[:], pD[:])
                            nc.sync.dma_start(out=d_out[4 * si + j],
                                              in_=fo[:])

                    def make_strip_units(si, p):
                        q0 = 512 * si
                        kb_max = 4 * (si + 1)
                        state = {"av": None}

                        def get_av():
                            if state["av"] is None:
                                state["av"] = psV.tile(
                                    [65, 1024], F32, tag="av",
                                    name=f"avps{si}_{p}")
                            return state["av"]

                        # units: fully-causal kb pairs (one exp per head),
                        # then the 4 diagonal blocks individually
                        units = [("pair", kb) for kb in range(0, 4 * si, 2)]
                        units += [("diag", kb) for kb in range(4 * si,
                                                               kb_max)]

                        def emit_s(unit):
                            kind, kb = unit
                            if kind == "pair":
                                ptbs = []
                                for hl in range(2):
                                    hb = 64 * hl
                                    sps = psS.tile(
                                        [128, 1024], F32, tag="sps",
                                        name=f"sp{si}_{p}_{kb}_{hl}")
                                    for dk in range(2):
                                        nc.tensor.matmul(
                                            sps[:, dk * 512:(dk + 1) * 512],
                                            kT[p][hb:hb + 64,
                                                  (kb + dk) * 128:
                                                  (kb + dk + 1) * 128],
                                            qT[p][hb:hb + 64, q0:q0 + 512],
                                            start=True, stop=True)
                                    ptb = ptp.tile(
                                        [128, 1024], BF16, tag="ptb",
                                        name=f"pt{si}_{p}_{kb}_{hl}")
                                    nc.scalar.activation(
                                        ptb[:], sps[:], AF.Exp, scale=SCALE)
                                    ptbs.append(ptb)
                                return ptbs
                            # diagonal block: both heads in one sps tile;
                            # the tri matmul adds -1e9 to future positions
                            # inside the psum group, so exp masks for free
                            o = 128 * kb - q0
                            L = 512 - o
                            sps = psS.tile([128, 1024], F32, tag="sps",
                                           name=f"sp{si}_{p}_{kb}")
                            for hl in range(2):
                                hb = 64 * hl
                                nc.tensor.matmul(
                                    sps[:, 512 * hl + o:512 * hl + 512],
                                    kT[p][hb:hb + 64,
                                          kb * 128:(kb + 1) * 128],
                                    qT[p][hb:hb + 64, q0 + o:q0 + 512],
                                    start=True, stop=False,
                                    skip_group_check=True)
                                nc.tensor.matmul(
                                    sps[:, 512 * hl + o:512 * hl + o + 128],
                                    ident[:], tri[:],
                                    start=False, stop=True,
                                    skip_group_check=True)
                            ptb = ptp.tile([128, 1024], BF16, tag="ptb",
                                           name=f"pt{si}_{p}_{kb}")
                            sps3 = sps[:].rearrange("a (h q) -> a h q", h=2)
                            ptb3 = ptb[:].rearrange("a (h q) -> a h q", h=2)
                            nc.scalar.activation(
                                ptb3[:, :, 0:L], sps3[:, :, o:512],
                                AF.Exp, scale=SCALE)
                            return ptb

                        def emit_av(unit, ptbs):
                            kind, kb = unit
                            av = get_av()
                            if kind == "pair":
                                for hl in range(2):
                                    for dk in range(2):
                                        nc.tensor.matmul(
                                            av[:, hl * 512:(hl + 1) * 512],
                                            va[2 * p + hl][
                                                :, (kb + dk) * 65:
                                                (kb + dk) * 65 + 65],
                                            ptbs[hl][:, dk * 512:
                                                     (dk + 1) * 512],
                                            start=(kb + dk == 0),
                                            stop=(kb + dk == kb_max - 1),
                                            skip_group_check=True)
                            else:
                                o = 128 * kb - q0
                                L = 512 - o
                                for hl in range(2):
                                    nc.tensor.matmul(
                                        av[:, hl * 512 + o:hl * 512 + 512],
                                        va[2 * p + hl][:, kb * 65:
                                                       kb * 65 + 65],
                                        ptbs[:, 512 * hl:512 * hl + L],
                                        start=(kb == 0),
                                        stop=(kb == kb_max - 1),
                                        skip_group_check=True)
                            if kind == "diag" and kb == kb_max - 1:
                                emit_div()

                        def emit_div():
                            # division, split per head to halve the chain
                            # latency: row 64 of av is the denominator
                            av = get_av()
                            for hl in range(2):
                                hsl = slice(hl * 512, (hl + 1) * 512)
                                r_sb = rp.tile([1, 512], F32, tag="r",
                                               name=f"rsb{si}_{p}_{hl}")
                                nc.vector.reciprocal(r_sb[:], av[64:65, hsl])
                                rb = rp.tile([64, 512], F32, tag="rb",
                                             name=f"rbb{si}_{p}_{hl}")
                                nc.gpsimd.partition_broadcast(rb[:], r_sb[:])
                                nc.vector.tensor_tensor(
                                    out=oT[p][64 * hl:64 * hl + 64,
                                              q0:q0 + 512],
                                    in0=av[0:64, hsl], in1=rb[:],
                                    op=ALU.mult)
                            divs_done[si] = divs_done.get(si, 0) + 1
                            if divs_done[si] == 2:
                                oproj_q.extend((si, j) for j in range(4))

                        return [(unit, emit_s, emit_av) for unit in units]

                    def mix(a, b):
                        # proportional interleave of two unit lists
                        out = []
                        ia = ib = 0
                        while ia < len(a) or ib < len(b):
                            if (ib < len(b)
                                    and ib * len(a) <= ia * len(b)):
                                out.append(b[ib])
                                ib += 1
                            elif ia < len(a):
                                out.append(a[ia])
                                ia += 1
                            else:
                                out.append(b[ib])
                                ib += 1
                        return out

                    # One continuous software pipeline over all strips. The
                    # all-diagonal (tiny-unit, latency-bound) strip-0 work is
                    # interleaved with strip 3's big units; the av psum ring
                    # (2 tiles) allows exactly two strips in flight. Each
                    # strip's out-projection chunks are sprinkled into later
                    # units to fill the PE while Act catches up on exp.
                    all_units = []
                    for si in [0, 3, 1, 2]:
                        for p in range(2):
                            all_units += make_strip_units(si, p)
                    pend = []
                    cnt = 0
                    for unit, s_fn, av_fn in all_units:
                        pend.append((av_fn, unit, s_fn(unit)))
                        if len(pend) > 4:
                            f, u, ptbs = pend.pop(0)
                            f(u, ptbs)
                        cnt += 1
                        # prefer filling the tiny diagonal units, where the
                        # PE otherwise starves behind exp latency
                        if len(oproj_q) > 10 and (unit[0] == "diag"
                                                 or cnt % 3 == 0):
                            emit_oproj_chunk(*oproj_q.pop(0))
                    while pend:
                        f, u, ptbs = pend.pop(0)
                        f(u, ptbs)
                    while oproj_q:
                        emit_oproj_chunk(*oproj_q.pop(0))

            if dbg:
                nc.sync.dma_start(out=d_dbg_q0[:], in_=qT[0][:])
                nc.sync.dma_start(out=d_dbg_k0[:], in_=kT[0][:])
                nc.sync.dma_start(out=d_dbg_va0[:], in_=va[0][:])
                nc.sync.dma_start(out=d_dbg_o0[:], in_=oT[0][:])

            psS.release()
            psV.release()

    nc.compile()
    return nc


_NC_CACHE = None


def _get_program():
    global _NC_CACHE
    if _NC_CACHE is None:
        _NC_CACHE = _build_program()
    return _NC_CACHE


def _rope_tables():
    inv_freq = 1.0 / (10000.0 ** (np.arange(0, HD, 2, dtype=np.float32) / HD))
    freqs = np.outer(np.arange(T, dtype=np.float32), inv_freq)  # [T, 32]
    emb = np.concatenate([freqs, freqs], axis=-1)               # [T, 64]
    return np.cos(emb), np.sin(emb)


def _to_bf16(a):
    import ml_dtypes
    return np.asarray(a, dtype=np.float32).astype(ml_dtypes.bfloat16)


def _host_prep(x, w_qkv, w_out):
    cos, sin = _rope_tables()          # [T, 64] each, original hd order
    # permuted + transposed tables [64, T], duplicated for a 2-head pair tile
    cosP = np.ascontiguousarray(cos.T[PI, :])                   # [64, T]
    sinP = sin.T[PI, :].copy()                                  # [64, T]
    sinP[0::2, :] *= -1.0                                       # sign baked in
    cos2 = _to_bf16(np.vstack([cosP, cosP]))
    sin2 = _to_bf16(np.vstack([sinP, sinP]))
    ident = _to_bf16(np.eye(128, dtype=np.float32))

    in_maps = []
    for core in range(NCORES):
        b = core // GROUPS
        h0 = (core % GROUPS) * HPC
        xT = np.ascontiguousarray(x[b].T)                       # [D, T]
        # column order: q pair0, k pair0, v pair0, v pair1, q pair1, k pair1
        cols = []
        for kind, p in [(0, 0), (1, 0)]:                        # q0, k0
            for hh in range(2):
                h = h0 + 2 * p + hh
                wcol = w_qkv[:, kind * D + h * HD:kind * D + (h + 1) * HD]
                cols.append(wcol[:, PI])
        for p in range(2):                                      # v (no perm)
            for hh in range(2):
                h = h0 + 2 * p + hh
                cols.append(w_qkv[:, 2 * D + h * HD:2 * D + (h + 1) * HD])
        for kind, p in [(0, 1), (1, 1)]:                        # q1, k1
            for hh in range(2):
                h = h0 + 2 * p + hh
                wcol = w_qkv[:, kind * D + h * HD:kind * D + (h + 1) * HD]
                cols.append(wcol[:, PI])
        w_cat = np.concatenate(cols, axis=1)                    # [D, 768]
        w_o = w_out[h0 * HD:(h0 + HPC) * HD, :]                 # [256, D]
        in_maps.append({
            "xT": _to_bf16(xT),
            "w_cat": _to_bf16(w_cat),
            "w_o": _to_bf16(w_o),
            "cos2": cos2,
            "sin2": sin2,
            "ident": ident,
        })
    return in_maps


def kernel(x, w_qkv, w_out):
    x = np.asarray(x, dtype=np.float32)
    w_qkv = np.asarray(w_qkv, dtype=np.float32)
    w_out = np.asarray(w_out, dtype=np.float32)
    nc = _get_program()
    in_maps = _host_prep(x, w_qkv, w_out)
    trace = bool(int(os.environ.get("KBENCH_TRACE", "0")))
    res = run_bass_kernel_spmd(nc, in_maps, list(range(NCORES)), trace=trace)
    if trace and res.exec_time_ns is not None:
        print(f"HW exec time: {res.exec_time_ns} ns")
    out = np.zeros((B, T, D), dtype=np.float32)
    for core in range(NCORES):
        b = core // GROUPS
        blk = res.results[core]["outp"].astype(np.float32)
        # (si, j, r, nn, q) -> rows (j,nn,r) = D, cols (si,q) = T
        dT = blk.reshape(4, 4, 128, 2, 512).transpose(1, 3, 2, 0, 4)
        out[b] += dT.reshape(D, T).T
    return out
